# revision 39
# baseline (speedup 1.0000x reference)
"""Trainium2 Bass kernel for ConvexDisplacementUpdate (B=4, L=4096, D=256).

new_coords = alpha * softmax(10 * qhat @ khat^T) @ coords + (1-alpha) * coords
q = l2norm(latents @ Wq^T), k = l2norm(latents @ Wk^T)  (row-wise l2norm)

Strategy (flash-attention style; the [L, L] score matrix never touches HBM):
  - 8 cores = (4 batches) x (2 query halves of 2048 rows). Host rolls each
    core's per-batch data so its own query rows are always columns 0:2048
    of the transposed latents -> one SPMD program, no per-core control flow.
  - Scores are computed transposed, S^T[m, l] = k_m . qhat_l, with k left
    UN-normalized; the per-m factor 10/||k_m|| is a per-partition scale
    folded into the exp() activation. qhat is scaled x16 so its fp8e4m3
    quantization stays out of the subnormal range; the /16 also folds into
    the exp scale.
  - q/k score operands are fp8e4m3 and the score matmul runs in DoubleRow
    mode: one matmul contracts the full D=256 via [128, 2, *] interleaved
    operands at 2 MACs/cell/cycle.
  - softmax without max-subtraction (|scores| <= 10, exp is safe in fp32).
  - numerator and denominator come from one PE matmul per (m-tile, l-block)
    with the ones-augmented coords [x, y, 1] as a [128, 6] hi|lo bf16
    stationary (tile_position packs 4 l-blocks in distinct PE col groups);
    hi+lo PSUM rows are summed on DVE at the end.
  - k production (projection, ssq, 10/16/||k|| scale) is software-pipelined
    INSIDE the phase-2 loop two m-blocks ahead, so the PE/ACT/DVE/Pool all
    stay busy instead of a serial k phase.
  - final alpha-blend + division happen on host (B*L*2 elements, trivial).
"""

import numpy as np

B, L, D = 4, 4096, 256
HALF = L // 2  # 2048 query rows per core
NCORES = 8
INV_TEMP = 10.0
QSCALE = 16.0  # qhat is stored as 16*qhat in fp8

_CACHE = {}


def build_module(reps=1, loop_n=0, qk_fp8=True):
    """Build + compile the SPMD Bass module (one program, 8 cores)."""
    from contextlib import ExitStack

    import concourse.bacc as bacc
    import concourse.mybir as mybir
    import concourse.tile as tile
    from concourse.bass import ts
    from concourse.masks import make_identity

    dt = mybir.dt
    f32 = dt.float32
    bf16 = dt.bfloat16
    AF = mybir.ActivationFunctionType
    ALU = mybir.AluOpType
    qdt = dt.float8e4 if qk_fp8 else bf16
    DR = mybir.MatmulPerfMode.DoubleRow

    nc = bacc.Bacc("TRN2", target_bir_lowering=False, debug=False,
                   num_devices=NCORES)

    latT = nc.dram_tensor("latT", [D, L], bf16, kind="ExternalInput")
    # wq/wk d-halves packed side by side: [128, 2*D] so one DMA loads each
    wqT_d = nc.dram_tensor("wqT", [128, 2 * D], bf16, kind="ExternalInput")
    wkT_d = nc.dram_tensor("wkT", [128, 2 * D], bf16, kind="ExternalInput")
    # fused per-m-tile stationary block: [hi_x hi_y hi_1 lo_x lo_y lo_1]
    caug6_d = nc.dram_tensor("caug6", [128, 6 * (L // 128)], bf16,
                             kind="ExternalInput")
    # host-precomputed norm scales: 16/||q_l||, (10/16)/||k_m||
    inv_q_d = nc.dram_tensor("inv_q", [128, HALF // 128], f32,
                             kind="ExternalInput")
    inv_k_d = nc.dram_tensor("inv_k", [128, L // 128], f32,
                             kind="ExternalInput")
    # hi+lo fold matrix: row c*4+lb of the fold output = pv rows
    # (32lb+c) + (32lb+3+c); built on host as 0/1 f32
    fold_d = nc.dram_tensor("fold", [128, 12], f32, kind="ExternalInput")
    # output rows indexed [c*4+lb] (c in {num_x, num_y, den}, lb l-block)
    pv_d = nc.dram_tensor("pv", [12, 512], f32, kind="ExternalOutput")

    NLT = L // 128        # 32 m-tiles
    NQT = HALF // 128     # 16 q l-tiles
    NMB = L // 512        # 8 m-blocks
    NLB = HALF // 512     # 4 l-blocks

    with tile.TileContext(nc) as tc:
        loop = tc.For_i(0, loop_n, 1) if loop_n else None
        if loop is not None:
            loop.__enter__()
        for _rep in range(reps):
            with ExitStack() as ctx:
                persist = ctx.enter_context(tc.tile_pool(name="persist", bufs=1))

                # ---- load inputs. Critical path first: wq + the first lat
                # half feed the q pipeline; wk/caug6/second lat half are
                # only needed later. ----
                wq_b = persist.tile([128, 2 * D], bf16, tag="wq_b")
                wk_b = persist.tile([128, 2 * D], bf16, tag="wk_b")
                wq = [wq_b[:, ts(i, D)] for i in range(2)]
                wk = [wk_b[:, ts(i, D)] for i in range(2)]
                lat = [persist.tile([128, L], bf16, tag=f"lat{i}", name=f"lat{i}") for i in range(2)]
                caug6 = persist.tile([128, 6 * NLT], bf16, tag="caug6")
                inv_q = persist.tile([128, NQT], f32, tag="inv_q")
                inv_kT = persist.tile([128, NLT], f32, tag="inv_kT")

                nc.sync.dma_start(out=wq_b, in_=wqT_d[:, :])
                for i in range(2):
                    nc.sync.dma_start(
                        out=lat[i][:, 0:1024],
                        in_=latT[i * 128:(i + 1) * 128, 0:1024])
                nc.sync.dma_start(out=inv_q, in_=inv_q_d[:, :])
                for i in range(2):
                    nc.sync.dma_start(
                        out=lat[i][:, 1024:2048],
                        in_=latT[i * 128:(i + 1) * 128, 1024:2048])
                nc.sync.dma_start(out=wk_b, in_=wkT_d[:, :])
                nc.sync.dma_start(out=inv_kT, in_=inv_k_d[:, :])
                nc.sync.dma_start(out=caug6, in_=caug6_d[:, :])
                fold = persist.tile([128, 12], dt.float32r, tag="fold")
                nc.sync.dma_start(out=fold,
                                  in_=fold_d[:, :].bitcast(dt.float32r))
                for i in range(2):
                    nc.sync.dma_start(
                        out=lat[i][:, 2048:4096],
                        in_=latT[i * 128:(i + 1) * 128, 2048:4096])

                ident = persist.tile([128, 128], bf16, tag="ident")
                make_identity(nc, ident)

                # persistent operand tiles for phase 2 (qT8 split per
                # l-block so the score stream starts before phase Q ends)
                qT8s = [persist.tile([128, 2, 512], qdt, tag=f"qT8_{lb}",
                                     name=f"qT8_{lb}") for lb in range(NLB)]
                kT8 = persist.tile([128, NLT, 2, 128], qdt, tag="kT8")

                # ---- phase Q: per-l-tile pipeline, no global barrier:
                # qle -> scaled bf16 qhat -> 2 PE transposes -> fp8 qT8 ----
                with ExitStack() as pq:
                    qle_ps = pq.enter_context(
                        tc.tile_pool(name="qle_ps", bufs=3, space="PSUM"))
                    tp_ps = pq.enter_context(
                        tc.tile_pool(name="tp_ps", bufs=3, space="PSUM"))
                    qh_pool = pq.enter_context(tc.tile_pool(name="qhat", bufs=3))
                    for lt in range(NQT):
                        qle = qle_ps.tile([128, D], f32, tag="qle")
                        nc.tensor.matmul(qle, lat[0][:, ts(lt, 128)], wq[0],
                                         start=True, stop=False)
                        nc.tensor.matmul(qle, lat[1][:, ts(lt, 128)], wq[1],
                                         start=False, stop=True)
                        qh = qh_pool.tile([128, D], bf16, tag="qh")
                        nc.scalar.mul(qh, qle, inv_q[:, lt:lt + 1])
                        tp = tp_ps.tile([128, 2, 128], bf16, tag="tp")
                        for et in range(2):
                            nc.tensor.transpose(tp[:, et, :],
                                                qh[:, ts(et, 128)], ident)
                        nc.vector.tensor_copy(
                            out=qT8s[lt // 4][:, :, ts(lt % 4, 128)], in_=tp)

                # ---- phase 2: k production pipelined 2 m-blocks ahead of
                # the score/exp/pv consumption loop ----
                with ExitStack() as p2:
                    sp_ps = p2.enter_context(
                        tc.tile_pool(name="sp_ps", bufs=2, space="PSUM"))
                    kp_ps = p2.enter_context(
                        tc.tile_pool(name="kp_ps", bufs=2, space="PSUM"))
                    pv_ps = p2.enter_context(
                        tc.tile_pool(name="pv_ps", bufs=1, space="PSUM"))
                    p_pool = p2.enter_context(tc.tile_pool(name="p_sb", bufs=5))
                    pv_all = pv_ps.tile([128, 512], f32, tag="pv")
                    # rows outside the 32lb..32lb+6 blocks are never written
                    # by the pv matmuls but are read by the fold chain
                    nc.vector.memset(pv_all, 0.0)

                    def emit_k(mb):
                        for et in range(2):
                            kp = kp_ps.tile([128, 4, 128], f32, tag="kp",
                                            name=f"kp{mb}_{et}")
                            nc.tensor.matmul(kp, wk[0][:, ts(et, 128)],
                                             lat[0][:, ts(mb, 512)],
                                             start=True, stop=False)
                            nc.tensor.matmul(kp, wk[1][:, ts(et, 128)],
                                             lat[1][:, ts(mb, 512)],
                                             start=False, stop=True)
                            nc.vector.tensor_copy(
                                out=kT8[:, 4 * mb:4 * mb + 4, et, :], in_=kp)

                    def emit_pv(t, ptiles):
                        for lb in range(NLB):
                            prhs = ptiles[lb // 2][:, ts(lb % 2, 512)]
                            nc.tensor.matmul(
                                pv_all[32 * lb:32 * lb + 6, :],
                                caug6[:, ts(t, 6)], prhs,
                                start=(t == 0), stop=(t == NLT - 1),
                                tile_position=(0, 32 * lb),
                                skip_group_check=True)

                    emit_k(0)
                    emit_k(1)
                    prev = None
                    for t in range(NLT):
                        if t % 4 == 0 and t // 4 + 2 < NMB:
                            emit_k(t // 4 + 2)
                        cur = []
                        for j in range(2):
                            sp = sp_ps.tile([128, 1024], f32, tag="sp")
                            for h in range(2):
                                lb = 2 * j + h
                                if qk_fp8:
                                    nc.tensor.matmul(
                                        sp[:, ts(h, 512)],
                                        kT8[:, t, :, :],
                                        qT8s[lb][:, :, :],
                                        start=True, stop=True, perf_mode=DR)
                                else:
                                    nc.tensor.matmul(
                                        sp[:, ts(h, 512)],
                                        kT8[:, t, 0, :],
                                        qT8s[lb][:, 0, :],
                                        start=True, stop=False)
                                    nc.tensor.matmul(
                                        sp[:, ts(h, 512)],
                                        kT8[:, t, 1, :],
                                        qT8s[lb][:, 1, :],
                                        start=False, stop=True)
                            p = p_pool.tile([128, 1024], bf16, tag="p")
                            nc.scalar.activation(p, sp, AF.Exp,
                                                 scale=inv_kT[:, t:t + 1])
                            cur.append(p)
                        if prev is not None:
                            emit_pv(t - 1, prev)
                        prev = cur
                    emit_pv(NLT - 1, prev)

                    out_sb = p2.enter_context(tc.tile_pool(name="out_sb", bufs=2))
                    fold_ps = p2.enter_context(
                        tc.tile_pool(name="fold_ps", bufs=1, space="PSUM"))
                    f32r = dt.float32r
                    pv_sb = out_sb.tile([128, 512], f32r, tag="pv_sb")
                    nc.vector.tensor_copy(out=pv_sb, in_=pv_all)
                    fps = fold_ps.tile([12, 512], f32, tag="fps")
                    nc.tensor.matmul(fps, fold, pv_sb, start=True, stop=True)
                    ot = out_sb.tile([12, 512], f32, tag="ot")
                    nc.vector.tensor_copy(out=ot, in_=fps)
                    nc.sync.dma_start(out=pv_d[:, :], in_=ot)

        if loop is not None:
            loop.__exit__(None, None, None)
    nc.compile()
    return nc


def _get_module():
    if "nc" not in _CACHE:
        _CACHE["nc"] = build_module()
    return _CACHE["nc"]


def make_in_maps(latents, current_coords, Wq, Wk):
    """Per-core input dicts. Core c -> batch c//2, query half c%2 (rolled
    so own query rows are always columns 0:2048)."""
    import ml_dtypes
    bf = ml_dtypes.bfloat16
    latents = np.asarray(latents, np.float32)
    coords = np.asarray(current_coords, np.float32)
    Wq = np.asarray(Wq, np.float32)
    Wk = np.asarray(Wk, np.float32)
    # [d, e] transposed weights, d-halves packed side by side: [128, 2*D]
    wqT = np.ascontiguousarray(
        Wq.T.reshape(2, 128, D).transpose(1, 0, 2).reshape(128, 2 * D).astype(bf))
    wkT = np.ascontiguousarray(
        Wk.T.reshape(2, 128, D).transpose(1, 0, 2).reshape(128, 2 * D).astype(bf))
    # host-side norm factors (match the device's bf16 operand rounding)
    latb = latents.astype(bf).astype(np.float32)
    q = np.einsum('bld,ed->ble', latb, Wq.astype(bf).astype(np.float32),
                  optimize=True)
    k = np.einsum('bld,ed->ble', latb, Wk.astype(bf).astype(np.float32),
                  optimize=True)
    inv_q_all = QSCALE / np.maximum(np.sqrt((q * q).sum(-1)), 1e-12)   # [B,L]
    inv_k_all = (INV_TEMP / QSCALE) / np.maximum(
        np.sqrt((k * k).sum(-1)), 1e-12)                               # [B,L]
    fold = np.zeros((128, 12), np.float32)
    for lb in range(4):
        for c in range(3):
            fold[32 * lb + c, c * 4 + lb] = 1.0
            fold[32 * lb + 3 + c, c * 4 + lb] = 1.0
    in_maps = []
    for c in range(NCORES):
        b, h = divmod(c, 2)
        lat_b = np.roll(latents[b], -HALF * h, axis=0)
        coo_b = np.roll(coords[b], -HALF * h, axis=0)
        invq_b = np.roll(inv_q_all[b], -HALF * h)[:HALF]
        invk_b = np.roll(inv_k_all[b], -HALF * h)
        aug = np.concatenate([coo_b, np.ones((L, 1), np.float32)], axis=1)
        caug = np.ascontiguousarray(
            aug.reshape(L // 128, 128, 3).transpose(1, 0, 2).reshape(128, -1))
        hi = caug.astype(bf)
        lo = (caug - hi.astype(np.float32)).astype(bf)
        # interleave into per-m-tile [hi_x hi_y hi_1 lo_x lo_y lo_1] blocks
        caug6 = np.concatenate(
            [hi.reshape(128, L // 128, 3), lo.reshape(128, L // 128, 3)],
            axis=2).reshape(128, -1)
        in_maps.append({
            "latT": np.ascontiguousarray(lat_b.T.astype(bf)),
            "wqT": wqT,
            "wkT": wkT,
            "caug6": np.ascontiguousarray(caug6),
            # [128, ntiles] layouts: row l of tile t sits at [l%128, t]
            "inv_q": np.ascontiguousarray(
                invq_b.reshape(HALF // 128, 128).T.astype(np.float32)),
            "inv_k": np.ascontiguousarray(
                invk_b.reshape(L // 128, 128).T.astype(np.float32)),
            "fold": fold,
        })
    return in_maps


def postprocess(results, current_coords, alpha):
    """Assemble (new_coords, displacement) from per-core pv = [num_x; num_y; den]."""
    coords = np.asarray(current_coords, np.float32)
    new_coords = np.empty((B, L, 2), np.float32)
    for c in range(NCORES):
        b, h = divmod(c, 2)
        pv = np.asarray(results[c]["pv"]).reshape(3, 4 * 512)  # rows c*4+lb
        wc = (pv[0:2, :] / pv[2:3, :]).T  # [2048, 2] = (W @ coords) rows
        rows = slice(h * HALF, (h + 1) * HALF)
        new_coords[b, rows] = alpha * wc + (1.0 - alpha) * coords[b, rows]
    displacement = new_coords - coords
    return new_coords, displacement


def kernel(latents, current_coords, Wq, Wk, alpha_raw, layer_idx=None):
    from concourse.bass_utils import run_bass_kernel_spmd

    nc = _get_module()
    in_maps = make_in_maps(latents, current_coords, Wq, Wk)
    res = run_bass_kernel_spmd(nc, in_maps, list(range(NCORES)))
    alpha = np.float32(1.0 / (1.0 + np.exp(-np.float64(np.asarray(alpha_raw)))))
    return postprocess(res.results, current_coords, alpha)


# revision 46
# speedup vs baseline: 1.9350x; 1.9350x over previous
"""Trainium2 Bass kernel for ConvexDisplacementUpdate (B=4, L=4096, D=256).

new_coords = alpha * softmax(10 * qhat @ khat^T) @ coords + (1-alpha) * coords
q = l2norm(latents @ Wq^T), k = l2norm(latents @ Wk^T)  (row-wise l2norm)

Strategy (flash-attention style; the [L, L] score matrix never touches HBM):
  - 8 cores = (4 batches) x (2 query halves of 2048 rows). Host rolls each
    core's per-batch data so its own query rows are always columns 0:2048
    of the transposed latents -> one SPMD program, no per-core control flow.
  - Scores are computed transposed, S^T[m, l] = k_m . qhat_l, with k left
    UN-normalized; the per-m factor 10/||k_m|| is a per-partition scale
    folded into the exp() activation. qhat is scaled x16 so its fp8e4m3
    quantization stays out of the subnormal range; the /16 also folds into
    the exp scale.
  - q/k score operands are fp8e4m3 and the score matmul runs in DoubleRow
    mode: one matmul contracts the full D=256 via [128, 2, *] interleaved
    operands at 2 MACs/cell/cycle.
  - softmax without max-subtraction (|scores| <= 10, exp is safe in fp32).
  - numerator and denominator come from one PE matmul per (m-tile, l-block)
    with the ones-augmented coords [x, y, 1] as a [128, 6] hi|lo bf16
    stationary (tile_position packs 4 l-blocks in distinct PE col groups);
    hi+lo PSUM rows are summed on DVE at the end.
  - k production (projection, ssq, 10/16/||k|| scale) is software-pipelined
    INSIDE the phase-2 loop two m-blocks ahead, so the PE/ACT/DVE/Pool all
    stay busy instead of a serial k phase.
  - final alpha-blend + division happen on host (B*L*2 elements, trivial).
"""

import numpy as np

B, L, D = 4, 4096, 256
HALF = L // 2  # 2048 query rows per core
NCORES = 8
INV_TEMP = 10.0
QSCALE = 16.0  # qhat is stored as 16*qhat in fp8

_CACHE = {}


def build_module(reps=1, loop_n=0, qk_fp8=True):
    """Build + compile the SPMD Bass module (one program, 8 cores)."""
    from contextlib import ExitStack

    import concourse.bacc as bacc
    import concourse.mybir as mybir
    import concourse.tile as tile
    from concourse.bass import ts
    from concourse.masks import make_identity

    dt = mybir.dt
    f32 = dt.float32
    bf16 = dt.bfloat16
    AF = mybir.ActivationFunctionType
    ALU = mybir.AluOpType
    qdt = dt.float8e4 if qk_fp8 else bf16
    DR = mybir.MatmulPerfMode.DoubleRow

    nc = bacc.Bacc("TRN2", target_bir_lowering=False, debug=False,
                   num_devices=NCORES)

    latT = nc.dram_tensor("latT", [D, L], bf16, kind="ExternalInput")
    # wq/wk d-halves packed side by side: [128, 2*D] so one DMA loads each
    wqT_d = nc.dram_tensor("wqT", [128, 2 * D], bf16, kind="ExternalInput")
    wkT_d = nc.dram_tensor("wkT", [128, 2 * D], bf16, kind="ExternalInput")
    # fused per-m-tile stationary block: [hi_x hi_y hi_1 lo_x lo_y lo_1]
    caug6_d = nc.dram_tensor("caug6", [128, 6 * (L // 128)], bf16,
                             kind="ExternalInput")
    # host-precomputed norm scales: 16/||q_l||, (10/16)/||k_m||
    inv_q_d = nc.dram_tensor("inv_q", [128, HALF // 128], f32,
                             kind="ExternalInput")
    inv_k_d = nc.dram_tensor("inv_k", [128, L // 128], f32,
                             kind="ExternalInput")
    # hi+lo fold matrix: row c*4+lb of the fold output = pv rows
    # (32lb+c) + (32lb+3+c); built on host as 0/1 f32
    fold_d = nc.dram_tensor("fold", [128, 12], f32, kind="ExternalInput")
    # output rows indexed [c*4+lb] (c in {num_x, num_y, den}, lb l-block)
    pv_d = nc.dram_tensor("pv", [12, 512], f32, kind="ExternalOutput")

    NLT = L // 128        # 32 m-tiles
    NQT = HALF // 128     # 16 q l-tiles
    NMB = L // 512        # 8 m-blocks
    NLB = HALF // 512     # 4 l-blocks

    with tile.TileContext(nc) as tc:
        loop = tc.For_i(0, loop_n, 1) if loop_n else None
        if loop is not None:
            loop.__enter__()
        for _rep in range(reps):
            with ExitStack() as ctx:
                persist = ctx.enter_context(tc.tile_pool(name="persist", bufs=1))

                # ---- load inputs. Critical path first: wq + the first lat
                # half feed the q pipeline; wk/caug6/second lat half are
                # only needed later. ----
                wq_b = persist.tile([128, 2 * D], bf16, tag="wq_b")
                wk_b = persist.tile([128, 2 * D], bf16, tag="wk_b")
                wq = [wq_b[:, ts(i, D)] for i in range(2)]
                wk = [wk_b[:, ts(i, D)] for i in range(2)]
                lat = [persist.tile([128, L], bf16, tag=f"lat{i}", name=f"lat{i}") for i in range(2)]
                caug6 = persist.tile([128, 6 * NLT], bf16, tag="caug6")
                inv_q = persist.tile([128, NQT], f32, tag="inv_q")
                inv_kT = persist.tile([128, NLT], f32, tag="inv_kT")

                nc.sync.dma_start(out=wq_b, in_=wqT_d[:, :])
                for i in range(2):
                    nc.sync.dma_start(
                        out=lat[i][:, 0:512],
                        in_=latT[i * 128:(i + 1) * 128, 0:512])
                nc.sync.dma_start(out=inv_q, in_=inv_q_d[:, :])
                for i in range(2):
                    nc.sync.dma_start(
                        out=lat[i][:, 512:2048],
                        in_=latT[i * 128:(i + 1) * 128, 512:2048])
                nc.sync.dma_start(out=wk_b, in_=wkT_d[:, :])
                nc.sync.dma_start(out=inv_kT, in_=inv_k_d[:, :])
                nc.sync.dma_start(out=caug6, in_=caug6_d[:, :])
                fold = persist.tile([128, 12], dt.float32r, tag="fold")
                nc.sync.dma_start(out=fold,
                                  in_=fold_d[:, :].bitcast(dt.float32r))
                for i in range(2):
                    nc.sync.dma_start(
                        out=lat[i][:, 2048:4096],
                        in_=latT[i * 128:(i + 1) * 128, 2048:4096])

                ident = persist.tile([128, 128], bf16, tag="ident")
                make_identity(nc, ident)

                # persistent operand tiles for phase 2 (qT8 split per
                # l-block so the score stream starts before phase Q ends)
                qT8s = [persist.tile([128, 2, 512], qdt, tag=f"qT8_{lb}",
                                     name=f"qT8_{lb}") for lb in range(NLB)]
                kT8 = persist.tile([128, NLT, 2, 128], qdt, tag="kT8")

                # ---- phase Q: per-l-tile pipeline, no global barrier:
                # qle -> scaled bf16 qhat -> 2 PE transposes -> fp8 qT8 ----
                with ExitStack() as pq:
                    qle_ps = pq.enter_context(
                        tc.tile_pool(name="qle_ps", bufs=3, space="PSUM"))
                    tp_ps = pq.enter_context(
                        tc.tile_pool(name="tp_ps", bufs=3, space="PSUM"))
                    qh_pool = pq.enter_context(tc.tile_pool(name="qhat", bufs=3))
                    for lt in range(NQT):
                        qle = qle_ps.tile([128, D], f32, tag="qle")
                        nc.tensor.matmul(qle, lat[0][:, ts(lt, 128)], wq[0],
                                         start=True, stop=False)
                        nc.tensor.matmul(qle, lat[1][:, ts(lt, 128)], wq[1],
                                         start=False, stop=True)
                        qh = qh_pool.tile([128, D], bf16, tag="qh")
                        nc.scalar.mul(qh, qle, inv_q[:, lt:lt + 1])
                        tp = tp_ps.tile([128, 2, 128], bf16, tag="tp")
                        for et in range(2):
                            nc.tensor.transpose(tp[:, et, :],
                                                qh[:, ts(et, 128)], ident)
                        nc.vector.tensor_copy(
                            out=qT8s[lt // 4][:, :, ts(lt % 4, 128)], in_=tp)

                # ---- phase 2: k production pipelined 2 m-blocks ahead of
                # the score/exp/pv consumption loop ----
                with ExitStack() as p2:
                    sp_ps = p2.enter_context(
                        tc.tile_pool(name="sp_ps", bufs=2, space="PSUM"))
                    kp_ps = p2.enter_context(
                        tc.tile_pool(name="kp_ps", bufs=2, space="PSUM"))
                    pv_ps = p2.enter_context(
                        tc.tile_pool(name="pv_ps", bufs=1, space="PSUM"))
                    p_pool = p2.enter_context(tc.tile_pool(name="p_sb", bufs=5))
                    pv_all = pv_ps.tile([128, 512], f32, tag="pv")
                    # rows outside the 32lb..32lb+6 blocks are never written
                    # by the pv matmuls but are read by the fold chain
                    nc.vector.memset(pv_all, 0.0)

                    def emit_k(mb):
                        for et in range(2):
                            kp = kp_ps.tile([128, 4, 128], f32, tag="kp",
                                            name=f"kp{mb}_{et}")
                            nc.tensor.matmul(kp, wk[0][:, ts(et, 128)],
                                             lat[0][:, ts(mb, 512)],
                                             start=True, stop=False)
                            nc.tensor.matmul(kp, wk[1][:, ts(et, 128)],
                                             lat[1][:, ts(mb, 512)],
                                             start=False, stop=True)
                            nc.vector.tensor_copy(
                                out=kT8[:, 4 * mb:4 * mb + 4, et, :], in_=kp)

                    def emit_pv(t, ptiles):
                        for lb in range(NLB):
                            prhs = ptiles[lb // 2][:, ts(lb % 2, 512)]
                            nc.tensor.matmul(
                                pv_all[32 * lb:32 * lb + 6, :],
                                caug6[:, ts(t, 6)], prhs,
                                start=(t == 0), stop=(t == NLT - 1),
                                tile_position=(0, 32 * lb),
                                skip_group_check=True)

                    def emit_scores(t, j):
                        sp = sp_ps.tile([128, 1024], f32, tag="sp")
                        for h in range(2):
                            lb = 2 * j + h
                            if qk_fp8:
                                nc.tensor.matmul(
                                    sp[:, ts(h, 512)],
                                    kT8[:, t, :, :],
                                    qT8s[lb][:, :, :],
                                    start=True, stop=True, perf_mode=DR)
                            else:
                                nc.tensor.matmul(
                                    sp[:, ts(h, 512)],
                                    kT8[:, t, 0, :],
                                    qT8s[lb][:, 0, :],
                                    start=True, stop=False)
                                nc.tensor.matmul(
                                    sp[:, ts(h, 512)],
                                    kT8[:, t, 1, :],
                                    qT8s[lb][:, 1, :],
                                    start=False, stop=True)
                        p = p_pool.tile([128, 1024], bf16, tag="p")
                        nc.scalar.activation(p, sp, AF.Exp,
                                             scale=inv_kT[:, t:t + 1])
                        return p

                    emit_k(0)
                    emit_k(1)
                    prev = None
                    for t in range(NLT):
                        if t % 4 == 0 and t // 4 + 2 < NMB:
                            emit_k(t // 4 + 2)
                        cur = [emit_scores(t, j) for j in range(2)]
                        if prev is not None:
                            emit_pv(t - 1, prev)
                        prev = cur
                    emit_pv(NLT - 1, prev)

                    out_sb = p2.enter_context(tc.tile_pool(name="out_sb", bufs=2))
                    f32r = dt.float32r
                    pv_sb = out_sb.tile([128, 512], f32r, tag="pv_sb")
                    nc.vector.tensor_copy(out=pv_sb, in_=pv_all)
                    # reuses a kp PSUM bank (same tag, same byte size)
                    fps = kp_ps.tile([12, 512], f32, tag="kp", name="fps")
                    nc.tensor.matmul(fps, fold, pv_sb, start=True, stop=True)
                    ot = out_sb.tile([12, 512], f32, tag="ot")
                    nc.vector.tensor_copy(out=ot, in_=fps)
                    nc.sync.dma_start(out=pv_d[:, :], in_=ot)

        if loop is not None:
            loop.__exit__(None, None, None)
    nc.compile()
    return nc


def _get_module():
    if "nc" not in _CACHE:
        _CACHE["nc"] = build_module()
    return _CACHE["nc"]


def make_in_maps(latents, current_coords, Wq, Wk):
    """Per-core input dicts. Core c -> batch c//2, query half c%2 (rolled
    so own query rows are always columns 0:2048)."""
    import ml_dtypes
    bf = ml_dtypes.bfloat16
    latents = np.asarray(latents, np.float32)
    coords = np.asarray(current_coords, np.float32)
    Wq = np.asarray(Wq, np.float32)
    Wk = np.asarray(Wk, np.float32)
    # [d, e] transposed weights, d-halves packed side by side: [128, 2*D]
    wqT = np.ascontiguousarray(
        Wq.T.reshape(2, 128, D).transpose(1, 0, 2).reshape(128, 2 * D).astype(bf))
    wkT = np.ascontiguousarray(
        Wk.T.reshape(2, 128, D).transpose(1, 0, 2).reshape(128, 2 * D).astype(bf))
    # host-side norm factors (match the device's bf16 operand rounding)
    latb = latents.astype(bf).astype(np.float32)
    q = np.einsum('bld,ed->ble', latb, Wq.astype(bf).astype(np.float32),
                  optimize=True)
    k = np.einsum('bld,ed->ble', latb, Wk.astype(bf).astype(np.float32),
                  optimize=True)
    inv_q_all = QSCALE / np.maximum(np.sqrt((q * q).sum(-1)), 1e-12)   # [B,L]
    inv_k_all = (INV_TEMP / QSCALE) / np.maximum(
        np.sqrt((k * k).sum(-1)), 1e-12)                               # [B,L]
    fold = np.zeros((128, 12), np.float32)
    for lb in range(4):
        for c in range(3):
            fold[32 * lb + c, c * 4 + lb] = 1.0
            fold[32 * lb + 3 + c, c * 4 + lb] = 1.0
    in_maps = []
    for c in range(NCORES):
        b, h = divmod(c, 2)
        lat_b = np.roll(latents[b], -HALF * h, axis=0)
        coo_b = np.roll(coords[b], -HALF * h, axis=0)
        invq_b = np.roll(inv_q_all[b], -HALF * h)[:HALF]
        invk_b = np.roll(inv_k_all[b], -HALF * h)
        aug = np.concatenate([coo_b, np.ones((L, 1), np.float32)], axis=1)
        caug = np.ascontiguousarray(
            aug.reshape(L // 128, 128, 3).transpose(1, 0, 2).reshape(128, -1))
        hi = caug.astype(bf)
        lo = (caug - hi.astype(np.float32)).astype(bf)
        # interleave into per-m-tile [hi_x hi_y hi_1 lo_x lo_y lo_1] blocks
        caug6 = np.concatenate(
            [hi.reshape(128, L // 128, 3), lo.reshape(128, L // 128, 3)],
            axis=2).reshape(128, -1)
        in_maps.append({
            "latT": np.ascontiguousarray(lat_b.T.astype(bf)),
            "wqT": wqT,
            "wkT": wkT,
            "caug6": np.ascontiguousarray(caug6),
            # [128, ntiles] layouts: row l of tile t sits at [l%128, t]
            "inv_q": np.ascontiguousarray(
                invq_b.reshape(HALF // 128, 128).T.astype(np.float32)),
            "inv_k": np.ascontiguousarray(
                invk_b.reshape(L // 128, 128).T.astype(np.float32)),
            "fold": fold,
        })
    return in_maps


def postprocess(results, current_coords, alpha):
    """Assemble (new_coords, displacement) from per-core pv = [num_x; num_y; den]."""
    coords = np.asarray(current_coords, np.float32)
    new_coords = np.empty((B, L, 2), np.float32)
    for c in range(NCORES):
        b, h = divmod(c, 2)
        pv = np.asarray(results[c]["pv"]).reshape(3, 4 * 512)  # rows c*4+lb
        wc = (pv[0:2, :] / pv[2:3, :]).T  # [2048, 2] = (W @ coords) rows
        rows = slice(h * HALF, (h + 1) * HALF)
        new_coords[b, rows] = alpha * wc + (1.0 - alpha) * coords[b, rows]
    displacement = new_coords - coords
    return new_coords, displacement


def kernel(latents, current_coords, Wq, Wk, alpha_raw, layer_idx=None):
    from concourse.bass_utils import run_bass_kernel_spmd

    nc = _get_module()
    in_maps = make_in_maps(latents, current_coords, Wq, Wk)
    res = run_bass_kernel_spmd(nc, in_maps, list(range(NCORES)))
    alpha = np.float32(1.0 / (1.0 + np.exp(-np.float64(np.asarray(alpha_raw)))))
    return postprocess(res.results, current_coords, alpha)
